# revision 11
# baseline (speedup 1.0000x reference)
import sys

sys.path.insert(0, "/opt/trn_rl_repo")

import math

import numpy as np
import ml_dtypes

import concourse.bass as bass
import concourse.mybir as mybir
import concourse.tile as tile
from concourse import bacc
from concourse.bass_utils import run_bass_kernel_spmd
from concourse.masks import make_identity

F32 = mybir.dt.float32
F32R = mybir.dt.float32r
BF16 = mybir.dt.bfloat16
EXPF = mybir.ActivationFunctionType.Exp
RECIPF = mybir.ActivationFunctionType.Reciprocal

B, S, D = 8, 1024, 1024
N_H = 16
REL_K = 16
d_k = D // N_H  # 64
N_CORES = 8
MASKVAL = -1e30
BF = np.dtype(ml_dtypes.bfloat16)

_CACHE = {}
TRACE = False


def build_module():
    nc = bacc.Bacc("TRN2", detect_race_conditions=False, num_swdge_queues=4)

    xT = nc.dram_tensor("xT", [D, S], BF16, kind="ExternalInput")
    Wqk = nc.dram_tensor("Wqk", [D, 2 * D], BF16, kind="ExternalInput")
    Wv = nc.dram_tensor("Wv", [D, D], BF16, kind="ExternalInput")
    Wp = nc.dram_tensor("Wp", [D, D], BF16, kind="ExternalInput")
    bqk = nc.dram_tensor("bqk", [128, 16], F32, kind="ExternalInput")
    bvrow = nc.dram_tensor("bvrow", [1, D], BF16, kind="ExternalInput")
    bprow = nc.dram_tensor("bprow", [1, D], BF16, kind="ExternalInput")
    dlut = nc.dram_tensor("dlut", [d_k, 16], BF16, kind="ExternalInput")
    dlv = nc.dram_tensor("dlv", [16, d_k], BF16, kind="ExternalInput")
    zbT = nc.dram_tensor("zbT", [128, 2561], BF16, kind="ExternalInput")
    OUT = nc.dram_tensor("OUT", [S, D], F32, kind="ExternalOutput")

    zbP = [nc.dram_tensor(f"zbP{k}", [128, 2561], BF16) for k in range(2)]
    ewP = [nc.dram_tensor(f"ewP{k}", [128, 1153], BF16) for k in range(4)]

    with tile.TileContext(nc) as tc:
        with (
            tc.tile_pool(name="pers", bufs=1) as pers,
            tc.tile_pool(name="mm", bufs=4, space="PSUM") as mmp,
        ):
            # ---- resident loads first, split across queues ----
            xT_sb = []
            for d in range(8):
                t = pers.tile([128, S], BF16, tag=f"xT{d}")
                eng = nc.sync if d % 2 == 0 else nc.scalar
                eng.dma_start(out=t[:], in_=xT[128 * d:128 * (d + 1), :])
                xT_sb.append(t)

            # ---- constants ----
            identf = pers.tile([128, 128], F32)
            make_identity(nc, identf[:])
            identb = pers.tile([128, 128], BF16)
            nc.vector.tensor_copy(identb[:], identf[:])
            dlut_sb = pers.tile([128, 16], BF16)
            nc.scalar.dma_start(out=dlut_sb[0:64, :], in_=dlut[:])
            nc.scalar.dma_start(out=dlut_sb[64:128, :], in_=dlut[:])
            dlv_sb = pers.tile([16, d_k], BF16)
            nc.scalar.dma_start(out=dlv_sb[:], in_=dlv[:])
            bqk_sb = pers.tile([128, 16], F32)
            nc.sync.dma_start(out=bqk_sb[:], in_=bqk[:])
            bv_sb = pers.tile([1, D], BF16)
            nc.sync.dma_start(out=bv_sb[:], in_=bvrow[:])
            bp_sb = pers.tile([1, D], BF16)
            nc.sync.dma_start(out=bp_sb[:], in_=bprow[:])
            ones_row = pers.tile([1, 512], BF16)
            nc.vector.memset(ones_row[:], 1.0)
            ones_col = pers.tile([1, 128], BF16)
            nc.vector.memset(ones_col[:], 1.0)

            # ---- v projection -> vhat_sb (65-stride layout + ones cols) ----
            vhat_sb = [pers.tile([128, 16 * 65], BF16, name=f"vh{jt}", tag=f"vh{jt}")
                       for jt in range(8)]
            with tc.tile_pool(name="wv", bufs=1) as wvp:
                # zbP init (template with causal mask / zeros); lives in the wv
                # pool so its teardown barrier lands after the v phase.
                zb_sb = wvp.tile([128, 2561], BF16)
                nc.sync.dma_start(out=zb_sb[:], in_=zbT[:])
                for k in range(2):
                    nc.sync.dma_start(out=bass.AP(tensor=zbP[k], offset=0,
                                                  ap=[[2561, 128], [1, 2561]]),
                                      in_=zb_sb[:])
                Wv_sb = []
                for d in range(8):
                    t = wvp.tile([128, D], BF16, tag=f"wv{d}")
                    nc.gpsimd.dma_start(out=t[:], in_=Wv[128 * d:128 * (d + 1), :])
                    Wv_sb.append(t)
                # Wqk on gpsimd, overlaps the v-projection compute
                Wqk_sb = []
                for d in range(8):
                    t = pers.tile([128, 2 * D], BF16, name=f"wqk{d}", tag=f"wqk{d}")
                    nc.gpsimd.dma_start(out=t[:], in_=Wqk[128 * d:128 * (d + 1), :])
                    Wqk_sb.append(t)
                for tt in range(8):
                    vt = vhat_sb[tt]
                    ones_ap = bass.AP(tensor=vt[:].tensor, offset=64,
                                      ap=[[16 * 65, 128], [65, 16]])
                    nc.vector.memset(ones_ap, 1.0)
                    for fc in range(2):
                        ps = mmp.tile([128, 512], F32, tag="mm")
                        for d in range(8):
                            nc.tensor.matmul(
                                ps[:],
                                xT_sb[d][:, 128 * tt:128 * (tt + 1)],
                                Wv_sb[d][:, 512 * fc:512 * (fc + 1)],
                                start=(d == 0), stop=False,
                            )
                        nc.tensor.matmul(
                            ps[:], ones_col[:],
                            bv_sb[:, 512 * fc:512 * (fc + 1)],
                            start=False, stop=True,
                        )
                        srcA = bass.AP(tensor=ps[:].tensor,
                                       offset=ps[:].offset,
                                       ap=[[512, 128], [64, 8], [1, 64]])
                        dst = bass.AP(tensor=vt[:].tensor, offset=65 * 8 * fc,
                                      ap=[[16 * 65, 128], [65, 8], [1, 64]])
                        nc.scalar.copy(dst, srcA)

            pair_sb = [pers.tile([128, S], BF16, name=f"pair{hp}", tag=f"pair{hp}")
                       for hp in range(8)]
            Wp_sb = [pers.tile([128, D], BF16, name=f"wp{d}", tag=f"wp{d}")
                     for d in range(8)]

            # dt strips: persistent, zero-initialized once; the diagonal write
            # covers the same cells every pair, untouched cells must stay 0.
            dt_tiles = [pers.tile([16, 1280], BF16, name=f"dt{h}", tag=f"dt{h}")
                        for h in range(2)]
            for h in range(2):
                nc.vector.memset(dt_tiles[h][:], 0.0)

            expT = pers.tile([128, 16 * 1024], BF16)
            # block-7 tail cols [128:144) per head are read by the ewP write but
            # never written by exps -> zero them once.
            nc.vector.memset(expT[:, 2048 * 7 + 128:2048 * 7 + 144], 0.0)
            nc.vector.memset(expT[:, 2048 * 7 + 1024 + 128:2048 * 7 + 1024 + 144], 0.0)

            # ---- attention ----
            with (
                tc.tile_pool(name="qk", bufs=2) as qkp,
                tc.tile_pool(name="dpp", bufs=2) as dpp,
                tc.tile_pool(name="bandp", bufs=2) as bandp,
                tc.tile_pool(name="eskp", bufs=2) as eskp,
                tc.tile_pool(name="denp", bufs=1) as denp,
                tc.tile_pool(name="pso", bufs=1, space="PSUM") as psop,
            ):
                def emit_qkproj_sec(hp, sec):
                    ftbase = 1024 * sec + 128 * hp
                    ft = 8 * sec + hp
                    dstt = qkp.tile([128, S], BF16, name=f"qk{sec}", tag=f"qk{sec}")
                    for tch in range(2):
                        ps = mmp.tile([128, 512], F32, tag="mm")
                        for d in range(8):
                            nc.tensor.matmul(
                                ps[:],
                                Wqk_sb[d][:, ftbase:ftbase + 128],
                                xT_sb[d][:, 512 * tch:512 * (tch + 1)],
                                start=(d == 0), stop=(d == 7),
                            )
                        nc.vector.tensor_scalar_add(
                            dstt[:, 512 * tch:512 * (tch + 1)], ps[:],
                            bqk_sb[:, ft:ft + 1])
                    return dstt

                def emit_band_part1(hp, qT_pair):
                    """dp strip matmuls + sheared dpSh DMA."""
                    dpT_h = []
                    for h in range(2):
                        dpT = dpp.tile([16, 1040], BF16, name=f"dpT{h}", tag=f"dpT{h}")
                        nc.vector.memset(dpT[:, 1024:1040], 0.0)
                        for tch in range(2):
                            psdp = mmp.tile([16, 512], F32, tag="mm")
                            nc.tensor.matmul(psdp[0:16, :],
                                             dlut_sb[64 * h:64 * h + 64, :],
                                             qT_pair[64 * h:64 * h + 64,
                                                     512 * tch:512 * (tch + 1)],
                                             start=True, stop=True)
                            nc.vector.tensor_copy(dpT[:, 512 * tch:512 * (tch + 1)],
                                                  psdp[0:16, :])
                        dpT_h.append(dpT)
                    dpSh = dpp.tile([32, 1024], BF16, tag="dpSh")
                    for h in range(2):
                        src = bass.AP(tensor=dpT_h[h][:].tensor, offset=0,
                                      ap=[[1041, 16], [1, 1024]])
                        nc.sync.dma_start(out=dpSh[16 * h:16 * h + 16, :], in_=src)
                    return dpSh

                def emit_band_part2(hp, dpSh):
                    """strip transposes + batched band write / masked read-back."""
                    dpS_all = dpp.tile([128, 256], BF16, tag="dpS")
                    for jt in range(8):
                        psd = mmp.tile([128, 32], BF16, tag="mm")
                        nc.tensor.matmul(psd[:, 0:32], dpSh[:, 128 * jt:128 * (jt + 1)],
                                         identb[0:32, 0:32], is_transpose=True,
                                         skip_group_check=True)
                        nc.vector.tensor_copy(dpS_all[:, 32 * jt:32 * (jt + 1)],
                                              psd[:, 0:32])
                    zb = zbP[hp % 2]
                    dstW = bass.AP(tensor=zb, offset=0,
                                   ap=[[2562, 128], [160, 16], [1, 16]])
                    srcW = bass.AP(tensor=dpS_all[:].tensor, offset=0,
                                   ap=[[256, 128], [16, 16], [1, 16]])
                    nc.scalar.dma_start(out=dstW, in_=srcW)
                    band = bandp.tile([128, 2560], BF16, tag="band")
                    srcR = bass.AP(tensor=zb, offset=0,
                                   ap=[[2561, 128], [1, 2560]])
                    nc.sync.dma_start(out=band[:], in_=srcR)
                    return band

                def emit_scores_head(hp, h, qT_pair, kT_pair, band, pso):
                    hg = 2 * hp + h
                    qT = qT_pair[64 * h:64 * h + 64, :]
                    kT = kT_pair[64 * h:64 * h + 64, :]
                    first = True
                    for jt in range(8):
                        j0 = 128 * jt
                        wd = min(512, S - j0)
                        win = min(144, S - j0)
                        ebase = 2048 * jt + 1024 * h
                        pss = mmp.tile([128, 512], F32, tag="mm")
                        nc.tensor.matmul(pss[:, 0:wd],
                                         kT[:, j0:j0 + 128],
                                         qT[:, j0:j0 + wd],
                                         start=True, stop=True)
                        nc.vector.tensor_add(
                            pss[:, 0:win], pss[:, 0:win],
                            band[:, 320 * jt + 160 * h:320 * jt + 160 * h + win])
                        nc.scalar.activation(expT[:, ebase:ebase + wd],
                                             pss[:, 0:wd], EXPF)
                        if S - j0 > 512:
                            w1 = S - j0 - 512
                            pss1 = mmp.tile([128, 512], F32, tag="mm")
                            nc.tensor.matmul(pss1[:, 0:w1],
                                             kT[:, j0:j0 + 128],
                                             qT[:, j0 + 512:S],
                                             start=True, stop=True)
                            nc.scalar.activation(expT[:, ebase + 512:ebase + 512 + w1],
                                                 pss1[:, 0:w1], EXPF)
                        lhs = vhat_sb[jt][:, 65 * hg:65 * hg + 65]
                        segs = ([(j0, 512), (512, 1024)] if j0 < 512
                                else [(j0, 1024)])
                        for (a, b2) in segs:
                            nc.tensor.matmul(pso[:, a:b2], lhs,
                                             expT[:, ebase + a - j0:ebase + b2 - j0],
                                             start=first, stop=False,
                                             skip_group_check=True)
                            first = False

                # prologue: pair 0 qk + band chain
                qk_q = emit_qkproj_sec(0, 0)
                dpSh_next = emit_band_part1(0, qk_q)
                qk_next = [qk_q, emit_qkproj_sec(0, 1)]
                band_next = emit_band_part2(0, dpSh_next)

                for hp in range(8):
                    qT_pair, kT_pair = qk_next
                    band = band_next
                    # prefetch: next pair q-proj + dp strips (before S2 so the
                    # band round-trip hides under this pair's compute)
                    if hp < 7:
                        qk_q = emit_qkproj_sec(hp + 1, 0)
                        dpSh_next = emit_band_part1(hp + 1, qk_q)
                    pso_h = [psop.tile([65, 1024], F32, name=f"pso{h}", tag=f"pso{h}")
                             for h in range(2)]
                    def emit_esk(h):
                        ew = ewP[2 * (hp % 2) + h]
                        dstE = bass.AP(tensor=ew, offset=0,
                                       ap=[[1152, 128], [144, 8], [1, 144]])
                        srcE = bass.AP(tensor=expT[:].tensor, offset=1024 * h,
                                       ap=[[16384, 128], [2048, 8], [1, 144]])
                        nc.scalar.dma_start(out=dstE, in_=srcE)
                        esk = eskp.tile([128, 128], BF16, name=f"esk{h}", tag=f"esk{h}")
                        srcK = bass.AP(tensor=ew, offset=0,
                                       ap=[[1153, 128], [144, 8], [1, 16]])
                        nc.sync.dma_start(out=esk[:], in_=srcK)
                        return esk

                    emit_scores_head(hp, 0, qT_pair, kT_pair, band, pso_h[0])
                    esk_h = [emit_esk(0)]
                    if hp < 7:
                        band_next = emit_band_part2(hp + 1, dpSh_next)
                    emit_scores_head(hp, 1, qT_pair, kT_pair, band, pso_h[1])
                    esk_h.append(emit_esk(1))
                    # k-proj for next pair covers the esk round-trip
                    if hp < 7:
                        qk_next = [qk_q, emit_qkproj_sec(hp + 1, 1)]
                    # reciprocal of the denominator rows (independent of dlv,
                    # which only touches pso rows 0:64) + psb broadcast: PE
                    # cover while the esk/dt round-trips land.
                    recip_h = []
                    for h in range(2):
                        rec = denp.tile([1, 1024], F32R, name=f"rec{h}", tag=f"rec{h}")
                        with nc.allow_low_precision(reason="denom recip broadcast"):
                            nc.vector.reciprocal(rec[:], pso_h[h][64:65, :])
                        recip_h.append(rec)
                    psb_c = []
                    for c in range(2):
                        psb = mmp.tile([128, 512], F32, tag="mm")
                        for h in range(2):
                            nc.tensor.matmul(psb[64 * h:64 * h + 64, :],
                                             ones_col[:, 0:64],
                                             recip_h[h][:, 512 * c:512 * (c + 1)],
                                             start=True, stop=True,
                                             skip_group_check=True)
                        psb_c.append(psb)

                    # S3b + S4a per head: transpose strip, dt write, dlv
                    def emit_dlv(h):
                        pso = pso_h[h]
                        for jt in range(8):
                            j0 = 128 * jt
                            win = min(144, S - j0)
                            a0, b0 = j0, j0 + win
                            dsegs = ([(a0, 512), (512, b0)] if (a0 < 512 < b0)
                                     else [(a0, b0)])
                            for (a, b2) in dsegs:
                                nc.tensor.matmul(
                                    pso[0:64, a:b2], dlv_sb[:],
                                    dt_tiles[h][:, 160 * jt + a - j0:160 * jt + b2 - j0],
                                    start=False,
                                    stop=(jt == 7 and (a, b2) == dsegs[-1]),
                                    skip_group_check=True)

                    for h in range(2):
                        psT = mmp.tile([128, 128], BF16, tag="mm")
                        nc.tensor.matmul(psT[:, :], esk_h[h][:], identb[:, :],
                                         is_transpose=True, skip_group_check=True)
                        eskT = eskp.tile([128, 128], BF16, name=f"eskT{h}",
                                         tag=f"eskT{h}")
                        nc.vector.tensor_copy(eskT[:], psT[:, :])
                        dstD = bass.AP(tensor=dt_tiles[h][:].tensor, offset=0,
                                       ap=[[1281, 16], [160, 8], [1, 128]])
                        srcD = bass.AP(tensor=eskT[:].tensor, offset=0,
                                       ap=[[128, 16], [2048, 8], [1, 128]])
                        eng = nc.sync if h == 0 else nc.scalar
                        eng.dma_start(out=dstD, in_=srcD)
                        emit_dlv(h)
                    # S4b: evict (Act, per column chunk) + normalize muls (DVE)
                    for c in range(2):
                        for h in range(2):
                            nc.scalar.copy(
                                pair_sb[hp][64 * h:64 * h + 64,
                                            512 * c:512 * (c + 1)],
                                pso_h[h][0:64, 512 * c:512 * (c + 1)])
                        nc.vector.tensor_mul(
                            pair_sb[hp][:, 512 * c:512 * (c + 1)],
                            pair_sb[hp][:, 512 * c:512 * (c + 1)], psb_c[c][:])
                    if hp == 3:
                        for d in range(8):
                            nc.gpsimd.dma_start(out=Wp_sb[d][:],
                                                in_=Wp[128 * d:128 * (d + 1), :])

            # ---- final projection ----
            with (
                tc.tile_pool(name="ps_p", bufs=2, space="PSUM") as ps_p,
                tc.tile_pool(name="outp", bufs=2) as outp,
            ):
                for tt in range(8):
                    ps = ps_p.tile([128, 1024], F32, tag="psp")
                    for fc in range(2):
                        for d in range(8):
                            nc.tensor.matmul(
                                ps[:, 512 * fc:512 * (fc + 1)],
                                pair_sb[d][:, 128 * tt:128 * (tt + 1)],
                                Wp_sb[d][:, 512 * fc:512 * (fc + 1)],
                                start=(d == 0), stop=False,
                            )
                        nc.tensor.matmul(
                            ps[:, 512 * fc:512 * (fc + 1)],
                            ones_col[:],
                            bp_sb[:, 512 * fc:512 * (fc + 1)],
                            start=False, stop=True,
                        )
                    ot = outp.tile([128, 1024], F32, tag="ot")
                    nc.vector.tensor_copy(ot[:], ps[:])
                    nc.sync.dma_start(out=OUT[128 * tt:128 * (tt + 1), :], in_=ot[:])

    nc.compile()
    return nc


def _host_prep(W_attn, b_attn, W_proj, b_proj, lut_k, lut_v):
    scale = 1.0 / math.sqrt(d_k)
    Wqk_h = np.concatenate([W_attn[:, :D], W_attn[:, D:2 * D] * scale], axis=1)
    bq = b_attn[:D]
    bk = b_attn[D:2 * D] * scale
    bqk_h = np.stack([np.concatenate([bq, bk])[128 * ft:128 * (ft + 1)]
                      for ft in range(16)], axis=1).astype(np.float32)
    bv_h = (b_attn[2 * D:3 * D] + np.tile(lut_v[0], N_H)).reshape(1, D)
    dlut_h = np.stack([(lut_k[16 - u] - lut_k[0]) * scale for u in range(16)],
                      axis=1)
    dlv_h = np.stack([lut_v[16 - u] - lut_v[0] for u in range(16)], axis=0)
    # zbP template: flat[2561*p + col], col = 160*jh + c (16 blocks of 160):
    # c < p -> MASKVAL (causal), c >= p+16 -> 0; band cells get overwritten.
    c_of_col = (np.arange(2561) % 160)[None, :]
    pvec = np.arange(128)[:, None]
    zbT_h = np.where(c_of_col < pvec, np.float32(MASKVAL), np.float32(0.0))
    return {
        "Wqk": Wqk_h.astype(BF),
        "Wv": np.ascontiguousarray(W_attn[:, 2 * D:3 * D]).astype(BF),
        "Wp": np.ascontiguousarray(W_proj).astype(BF),
        "bqk": bqk_h,
        "bvrow": bv_h.astype(BF),
        "bprow": np.asarray(b_proj).reshape(1, D).astype(BF),
        "dlut": dlut_h.astype(BF),
        "dlv": dlv_h.astype(BF),
        "zbT": zbT_h.astype(BF),
    }


def kernel(x, W_attn, b_attn, W_proj, b_proj, lut_k, lut_v):
    x = np.asarray(x, np.float32)
    shared = _host_prep(np.asarray(W_attn, np.float32),
                        np.asarray(b_attn, np.float32),
                        np.asarray(W_proj, np.float32),
                        np.asarray(b_proj, np.float32),
                        np.asarray(lut_k, np.float32),
                        np.asarray(lut_v, np.float32))
    if "nc" not in _CACHE:
        _CACHE["nc"] = build_module()
    nc = _CACHE["nc"]
    in_maps = []
    for b in range(N_CORES):
        m = dict(shared)
        m["xT"] = np.ascontiguousarray(x[b].T).astype(BF)
        in_maps.append(m)
    res = run_bass_kernel_spmd(nc, in_maps, list(range(N_CORES)), trace=TRACE)
    _CACHE["last_result"] = res
    out = np.stack([res.results[b]["OUT"] for b in range(N_CORES)], axis=0)
    return out.astype(np.float32)


# revision 18
# speedup vs baseline: 1.7716x; 1.7716x over previous
import sys

sys.path.insert(0, "/opt/trn_rl_repo")

import math

import numpy as np
import ml_dtypes

import concourse.bass as bass
import concourse.mybir as mybir
import concourse.tile as tile
from concourse import bacc
from concourse.bass_utils import run_bass_kernel_spmd
from concourse.masks import make_identity

F32 = mybir.dt.float32
F32R = mybir.dt.float32r
BF16 = mybir.dt.bfloat16
EXPF = mybir.ActivationFunctionType.Exp
RECIPF = mybir.ActivationFunctionType.Reciprocal

B, S, D = 8, 1024, 1024
N_H = 16
REL_K = 16
d_k = D // N_H  # 64
N_CORES = 8
MASKVAL = -1e30
BF = np.dtype(ml_dtypes.bfloat16)

_CACHE = {}
TRACE = False


def build_module():
    nc = bacc.Bacc("TRN2", detect_race_conditions=False, num_swdge_queues=4)

    xT = nc.dram_tensor("xT", [D, S], BF16, kind="ExternalInput")
    Wqk = nc.dram_tensor("Wqk", [D, 2 * D], BF16, kind="ExternalInput")
    Wv = nc.dram_tensor("Wv", [D, D], BF16, kind="ExternalInput")
    Wp = nc.dram_tensor("Wp", [D, D], BF16, kind="ExternalInput")
    bqk = nc.dram_tensor("bqk", [128, 16], F32, kind="ExternalInput")
    bvrow = nc.dram_tensor("bvrow", [1, D], BF16, kind="ExternalInput")
    bprow = nc.dram_tensor("bprow", [1, D], BF16, kind="ExternalInput")
    dlut = nc.dram_tensor("dlut", [d_k, 16], BF16, kind="ExternalInput")
    dlv = nc.dram_tensor("dlv", [16, d_k], BF16, kind="ExternalInput")
    zbT = nc.dram_tensor("zbT", [128, 2561], BF16, kind="ExternalInput")
    OUT = nc.dram_tensor("OUT", [S, D], F32, kind="ExternalOutput")

    zbP = [nc.dram_tensor(f"zbP{k}", [128, 2561], BF16) for k in range(2)]
    dtD = [nc.dram_tensor(f"dtD{k}", [128, 160], BF16) for k in range(4)]
    ewP = [nc.dram_tensor(f"ewP{k}", [128, 1153], BF16) for k in range(4)]

    with tile.TileContext(nc) as tc:
        with (
            tc.tile_pool(name="pers", bufs=1) as pers,
            tc.tile_pool(name="mm", bufs=4, space="PSUM") as mmp,
        ):
            # ---- resident loads first, split across queues ----
            xT_sb = []
            for d in range(8):
                t = pers.tile([128, S], BF16, tag=f"xT{d}")
                eng = nc.sync if d % 2 == 0 else nc.scalar
                eng.dma_start(out=t[:], in_=xT[128 * d:128 * (d + 1), :])
                xT_sb.append(t)

            # ---- constants ----
            identf = pers.tile([128, 128], F32)
            make_identity(nc, identf[:])
            identb = pers.tile([128, 128], BF16)
            nc.vector.tensor_copy(identb[:], identf[:])
            dlut_sb = pers.tile([128, 16], BF16)
            nc.scalar.dma_start(out=dlut_sb[0:64, :], in_=dlut[:])
            nc.scalar.dma_start(out=dlut_sb[64:128, :], in_=dlut[:])
            dlv_sb = pers.tile([16, d_k], BF16)
            nc.scalar.dma_start(out=dlv_sb[:], in_=dlv[:])
            bqk_sb = pers.tile([128, 16], F32)
            nc.sync.dma_start(out=bqk_sb[:], in_=bqk[:])
            bv_sb = pers.tile([1, D], BF16)
            nc.sync.dma_start(out=bv_sb[:], in_=bvrow[:])
            bp_sb = pers.tile([1, D], BF16)
            nc.sync.dma_start(out=bp_sb[:], in_=bprow[:])
            ones_row = pers.tile([1, 512], BF16)
            nc.vector.memset(ones_row[:], 1.0)
            ones_col = pers.tile([1, 128], BF16)
            nc.vector.memset(ones_col[:], 1.0)
            ones_rf = pers.tile([1, 64], F32)
            nc.vector.memset(ones_rf[:], 1.0)
            ones_r = pers.tile([1, 64], F32R)
            nc.vector.tensor_copy(ones_r[:], ones_rf[:])

            # ---- v projection -> vhat_sb (65-stride layout + ones cols) ----
            vhat_sb = [pers.tile([128, 16 * 65], BF16, name=f"vh{jt}", tag=f"vh{jt}")
                       for jt in range(8)]
            with tc.tile_pool(name="wv", bufs=1) as wvp:
                # zbP init (template with causal mask / zeros); lives in the wv
                # pool so its teardown barrier lands after the v phase.
                zb_sb = wvp.tile([128, 2561], BF16)
                nc.sync.dma_start(out=zb_sb[:], in_=zbT[:])
                for k in range(2):
                    nc.sync.dma_start(out=bass.AP(tensor=zbP[k], offset=0,
                                                  ap=[[2561, 128], [1, 2561]]),
                                      in_=zb_sb[:])
                Wv_sb = []
                for d in range(8):
                    t = wvp.tile([128, D], BF16, tag=f"wv{d}")
                    nc.gpsimd.dma_start(out=t[:], in_=Wv[128 * d:128 * (d + 1), :])
                    Wv_sb.append(t)
                # Wqk on gpsimd, overlaps the v-projection compute
                Wqk_sb = []
                for d in range(8):
                    t = pers.tile([128, 2 * D], BF16, name=f"wqk{d}", tag=f"wqk{d}")
                    nc.gpsimd.dma_start(out=t[:], in_=Wqk[128 * d:128 * (d + 1), :])
                    Wqk_sb.append(t)
                for tt in range(8):
                    vt = vhat_sb[tt]
                    ones_ap = bass.AP(tensor=vt[:].tensor, offset=64,
                                      ap=[[16 * 65, 128], [65, 16]])
                    nc.vector.memset(ones_ap, 1.0)
                    for fc in range(2):
                        ps = mmp.tile([128, 512], F32, tag="mm")
                        for d in range(8):
                            nc.tensor.matmul(
                                ps[:],
                                xT_sb[d][:, 128 * tt:128 * (tt + 1)],
                                Wv_sb[d][:, 512 * fc:512 * (fc + 1)],
                                start=(d == 0), stop=False,
                            )
                        nc.tensor.matmul(
                            ps[:], ones_col[:],
                            bv_sb[:, 512 * fc:512 * (fc + 1)],
                            start=False, stop=True,
                        )
                        srcA = bass.AP(tensor=ps[:].tensor,
                                       offset=ps[:].offset,
                                       ap=[[512, 128], [64, 8], [1, 64]])
                        dst = bass.AP(tensor=vt[:].tensor, offset=65 * 8 * fc,
                                      ap=[[16 * 65, 128], [65, 8], [1, 64]])
                        nc.scalar.copy(dst, srcA)

            pair_sb = [pers.tile([128, S], BF16, name=f"pair{hp}", tag=f"pair{hp}")
                       for hp in range(8)]
            Wp_sb = [pers.tile([128, D], BF16, name=f"wp{d}", tag=f"wp{d}")
                     for d in range(8)]

            # dt strips go through DRAM: the diagonal write covers the same
            # cells every pair; the zero background persists in DRAM.
            dtz = pers.tile([128, 160], BF16)
            nc.vector.memset(dtz[:], 0.0)
            for k2 in range(4):
                nc.gpsimd.dma_start(out=dtD[k2][:], in_=dtz[:])

            expT = pers.tile([128, 16 * 1024], BF16)
            # block-7 tail cols [128:144) per head are read by the ewP write but
            # never written by exps -> zero them once.
            nc.vector.memset(expT[:, 2048 * 7 + 128:2048 * 7 + 144], 0.0)
            nc.vector.memset(expT[:, 2048 * 7 + 1024 + 128:2048 * 7 + 1024 + 144], 0.0)

            # ---- attention ----
            with (
                tc.tile_pool(name="qk", bufs=2) as qkp,
                tc.tile_pool(name="dpp", bufs=2) as dpp,
                tc.tile_pool(name="bandp", bufs=2) as bandp,
                tc.tile_pool(name="eskp", bufs=2) as eskp,
                tc.tile_pool(name="denp", bufs=1) as denp,
                tc.tile_pool(name="pso", bufs=1, space="PSUM") as psop,
            ):
                def emit_qkproj_sec(hp, sec):
                    ftbase = 1024 * sec + 128 * hp
                    ft = 8 * sec + hp
                    dstt = qkp.tile([128, S], BF16, name=f"qk{sec}", tag=f"qk{sec}")
                    for tch in range(2):
                        ps = mmp.tile([128, 512], F32, tag="mm")
                        for d in range(8):
                            nc.tensor.matmul(
                                ps[:],
                                Wqk_sb[d][:, ftbase:ftbase + 128],
                                xT_sb[d][:, 512 * tch:512 * (tch + 1)],
                                start=(d == 0), stop=(d == 7),
                            )
                        nc.vector.tensor_scalar_add(
                            dstt[:, 512 * tch:512 * (tch + 1)], ps[:],
                            bqk_sb[:, ft:ft + 1])
                    return dstt

                def emit_band_part1(hp, qT_pair):
                    """dp strip matmuls + sheared dpSh DMA."""
                    dpT_h = []
                    for h in range(2):
                        dpT = dpp.tile([16, 1040], BF16, name=f"dpT{h}", tag=f"dpT{h}")
                        nc.vector.memset(dpT[:, 1024:1040], 0.0)
                        for tch in range(2):
                            psdp = mmp.tile([16, 512], F32, tag="mm")
                            nc.tensor.matmul(psdp[0:16, :],
                                             dlut_sb[64 * h:64 * h + 64, :],
                                             qT_pair[64 * h:64 * h + 64,
                                                     512 * tch:512 * (tch + 1)],
                                             start=True, stop=True)
                            nc.vector.tensor_copy(dpT[:, 512 * tch:512 * (tch + 1)],
                                                  psdp[0:16, :])
                        dpT_h.append(dpT)
                    dpSh = dpp.tile([32, 1024], BF16, tag="dpSh")
                    for h in range(2):
                        src = bass.AP(tensor=dpT_h[h][:].tensor, offset=0,
                                      ap=[[1041, 16], [1, 1024]])
                        nc.sync.dma_start(out=dpSh[16 * h:16 * h + 16, :], in_=src)
                    return dpSh

                def emit_band_part2(hp, dpSh):
                    """strip transposes + batched band write / masked read-back."""
                    dpS_all = dpp.tile([128, 256], BF16, tag="dpS")
                    for jt in range(8):
                        psd = mmp.tile([128, 32], BF16, tag="mm")
                        nc.tensor.matmul(psd[:, 0:32], dpSh[:, 128 * jt:128 * (jt + 1)],
                                         identb[0:32, 0:32], is_transpose=True,
                                         skip_group_check=True)
                        nc.vector.tensor_copy(dpS_all[:, 32 * jt:32 * (jt + 1)],
                                              psd[:, 0:32])
                    zb = zbP[hp % 2]
                    dstW = bass.AP(tensor=zb, offset=0,
                                   ap=[[2562, 128], [160, 16], [1, 16]])
                    srcW = bass.AP(tensor=dpS_all[:].tensor, offset=0,
                                   ap=[[256, 128], [16, 16], [1, 16]])
                    nc.scalar.dma_start(out=dstW, in_=srcW)
                    band = bandp.tile([128, 2560], BF16, tag="band")
                    srcR = bass.AP(tensor=zb, offset=0,
                                   ap=[[2561, 128], [1, 2560]])
                    nc.sync.dma_start(out=band[:], in_=srcR)
                    return band

                def emit_scores_head(hp, h, qT_pair, kT_pair, band, pso):
                    hg = 2 * hp + h
                    qT = qT_pair[64 * h:64 * h + 64, :]
                    kT = kT_pair[64 * h:64 * h + 64, :]
                    for jt in range(8):
                        j0 = 128 * jt
                        wd = min(512, S - j0)
                        win = min(144, S - j0)
                        ebase = 2048 * jt + 1024 * h
                        pss = mmp.tile([128, 512], F32, tag="mm")
                        nc.tensor.matmul(pss[:, 0:wd],
                                         kT[:, j0:j0 + 128],
                                         qT[:, j0:j0 + wd],
                                         start=True, stop=True)
                        nc.vector.tensor_add(
                            pss[:, 0:win], pss[:, 0:win],
                            band[:, 320 * jt + 160 * h:320 * jt + 160 * h + win])
                        nc.scalar.activation(expT[:, ebase:ebase + wd],
                                             pss[:, 0:wd], EXPF)
                        if S - j0 > 512:
                            w1 = S - j0 - 512
                            pss1 = mmp.tile([128, 512], F32, tag="mm")
                            nc.tensor.matmul(pss1[:, 0:w1],
                                             kT[:, j0:j0 + 128],
                                             qT[:, j0 + 512:S],
                                             start=True, stop=True)
                            nc.scalar.activation(expT[:, ebase + 512:ebase + 512 + w1],
                                                 pss1[:, 0:w1], EXPF)
                        lhs = vhat_sb[jt][:, 65 * hg:65 * hg + 65]
                        segs = ([(j0, 512), (512, 1024)] if j0 < 512
                                else [(j0, 1024)])
                        for (a, b2) in segs:
                            nc.tensor.matmul(pso[:, a:b2], lhs,
                                             expT[:, ebase + a - j0:ebase + b2 - j0],
                                             start=(jt == 0), stop=False,
                                             skip_group_check=True)

                # prologue: pair 0 qk + band chain
                qk_q = emit_qkproj_sec(0, 0)
                dpSh_next = emit_band_part1(0, qk_q)
                qk_next = [qk_q, emit_qkproj_sec(0, 1)]
                band_next = emit_band_part2(0, dpSh_next)

                dt_tiles = [None, None]
                for hp in range(8):
                    qT_pair, kT_pair = qk_next
                    band = band_next
                    # prefetch: next pair q-proj + dp strips (before S2 so the
                    # band round-trip hides under this pair's compute)
                    if hp < 7:
                        qk_q = emit_qkproj_sec(hp + 1, 0)
                        dpSh_next = emit_band_part1(hp + 1, qk_q)
                    pso_h = [psop.tile([65, 1024], F32, name=f"pso{h}", tag=f"pso{h}")
                             for h in range(2)]
                    def emit_esk(h):
                        ew = ewP[2 * (hp % 2) + h]
                        dstE = bass.AP(tensor=ew, offset=0,
                                       ap=[[1152, 128], [144, 8], [1, 144]])
                        srcE = bass.AP(tensor=expT[:].tensor, offset=1024 * h,
                                       ap=[[16384, 128], [2048, 8], [1, 144]])
                        nc.scalar.dma_start(out=dstE, in_=srcE)
                        esk = eskp.tile([128, 128], BF16, name=f"esk{h}", tag=f"esk{h}")
                        srcK = bass.AP(tensor=ew, offset=0,
                                       ap=[[1153, 128], [144, 8], [1, 16]])
                        nc.sync.dma_start(out=esk[:], in_=srcK)
                        return esk

                    emit_scores_head(hp, 0, qT_pair, kT_pair, band, pso_h[0])
                    esk_h = [emit_esk(0)]
                    if hp < 7:
                        band_next = emit_band_part2(hp + 1, dpSh_next)
                    emit_scores_head(hp, 1, qT_pair, kT_pair, band, pso_h[1])
                    esk_h.append(emit_esk(1))
                    # k-proj for next pair covers the esk round-trip
                    if hp < 7:
                        qk_next = [qk_q, emit_qkproj_sec(hp + 1, 1)]
                    # reciprocal of the denominator rows (independent of dlv,
                    # which only touches pso rows 0:64)
                    recip_h = []
                    for h in range(2):
                        rec = denp.tile([1, 1024], F32R, name=f"rec{h}", tag=f"rec{h}")
                        with nc.allow_low_precision(reason="denom recip broadcast"):
                            nc.vector.reciprocal(rec[:], pso_h[h][64:65, :])
                        recip_h.append(rec)

                    def emit_dlv(h):
                        pso = pso_h[h]
                        for jt in range(8):
                            j0 = 128 * jt
                            win = min(144, S - j0)
                            a0, b0 = j0, j0 + win
                            dsegs = ([(a0, 512), (512, b0)] if (a0 < 512 < b0)
                                     else [(a0, b0)])
                            for (a, b2) in dsegs:
                                nc.tensor.matmul(
                                    pso[0:64, a:b2], dlv_sb[:],
                                    dt_tiles[h][:, 160 * jt + a - j0:160 * jt + b2 - j0],
                                    start=False,
                                    stop=(jt == 7 and (a, b2) == dsegs[-1]),
                                    skip_group_check=True)

                    # per-head tail: strip transpose -> dt write -> recip
                    # broadcast -> dlv -> evict -> normalize mul
                    for h in range(2):
                        psT = mmp.tile([128, 128], BF16, tag="mm")
                        nc.tensor.matmul(psT[:, :], esk_h[h][:], identb[:, :],
                                         is_transpose=True, skip_group_check=True)
                        eskT = eskp.tile([128, 128], BF16, name=f"eskT{h}",
                                         tag=f"eskT{h}")
                        nc.vector.tensor_copy(eskT[:], psT[:, :])
                        dtd = dtD[2 * (hp % 2) + h]
                        # plain pitched rect write; the +u shear happens on the
                        # 159-stride read-back (DRAM side, arbitrary strides)
                        eng = nc.sync if h == 0 else nc.scalar
                        eng.dma_start(out=bass.AP(tensor=dtd, offset=0,
                                                  ap=[[160, 128], [1, 128]]),
                                      in_=eskT[:])
                        dt_t = eskp.tile([16, 1280], BF16, name=f"dt{h}",
                                         tag=f"dt{h}")
                        srcR = bass.AP(tensor=dtd, offset=0,
                                       ap=[[159, 16], [2560, 8], [1, 160]])
                        dstR = bass.AP(tensor=dt_t[:].tensor, offset=0,
                                       ap=[[1280, 16], [160, 8], [1, 160]])
                        eng.dma_start(out=dstR, in_=srcR)
                        dt_tiles[h] = dt_t
                        psb_cs = []
                        for c in range(2):
                            psb = mmp.tile([64, 512], F32, tag="mm")
                            nc.tensor.matmul(psb[0:64, :],
                                             ones_r[:],
                                             recip_h[h][:, 512 * c:512 * (c + 1)],
                                             start=True, stop=True)
                            psb_cs.append(psb)
                        emit_dlv(h)
                        for c in range(2):
                            nc.scalar.copy(
                                pair_sb[hp][64 * h:64 * h + 64,
                                            512 * c:512 * (c + 1)],
                                pso_h[h][0:64, 512 * c:512 * (c + 1)])
                            nc.vector.tensor_mul(
                                pair_sb[hp][64 * h:64 * h + 64,
                                            512 * c:512 * (c + 1)],
                                pair_sb[hp][64 * h:64 * h + 64,
                                            512 * c:512 * (c + 1)],
                                psb_cs[c][0:64, :])
                    if hp == 3:
                        for d in range(8):
                            nc.gpsimd.dma_start(out=Wp_sb[d][:],
                                                in_=Wp[128 * d:128 * (d + 1), :])

            # ---- final projection ----
            with (
                tc.tile_pool(name="ps_p", bufs=2, space="PSUM") as ps_p,
                tc.tile_pool(name="outp", bufs=2) as outp,
            ):
                for tt in range(8):
                    ps = ps_p.tile([128, 1024], F32, tag="psp")
                    for fc in range(2):
                        for d in range(8):
                            nc.tensor.matmul(
                                ps[:, 512 * fc:512 * (fc + 1)],
                                pair_sb[d][:, 128 * tt:128 * (tt + 1)],
                                Wp_sb[d][:, 512 * fc:512 * (fc + 1)],
                                start=(d == 0), stop=False,
                            )
                        nc.tensor.matmul(
                            ps[:, 512 * fc:512 * (fc + 1)],
                            ones_col[:],
                            bp_sb[:, 512 * fc:512 * (fc + 1)],
                            start=False, stop=True,
                        )
                    ot = outp.tile([128, 1024], F32, tag="ot")
                    nc.vector.tensor_copy(ot[:], ps[:])
                    nc.sync.dma_start(out=OUT[128 * tt:128 * (tt + 1), :], in_=ot[:])

    nc.compile()
    return nc


def _host_prep(W_attn, b_attn, W_proj, b_proj, lut_k, lut_v):
    scale = 1.0 / math.sqrt(d_k)
    Wqk_h = np.concatenate([W_attn[:, :D], W_attn[:, D:2 * D] * scale], axis=1)
    bq = b_attn[:D]
    bk = b_attn[D:2 * D] * scale
    bqk_h = np.stack([np.concatenate([bq, bk])[128 * ft:128 * (ft + 1)]
                      for ft in range(16)], axis=1).astype(np.float32)
    bv_h = (b_attn[2 * D:3 * D] + np.tile(lut_v[0], N_H)).reshape(1, D)
    dlut_h = np.stack([(lut_k[16 - u] - lut_k[0]) * scale for u in range(16)],
                      axis=1)
    dlv_h = np.stack([lut_v[16 - u] - lut_v[0] for u in range(16)], axis=0)
    # zbP template: flat[2561*p + col], col = 160*jh + c (16 blocks of 160):
    # c < p -> MASKVAL (causal), c >= p+16 -> 0; band cells get overwritten.
    c_of_col = (np.arange(2561) % 160)[None, :]
    pvec = np.arange(128)[:, None]
    zbT_h = np.where(c_of_col < pvec, np.float32(MASKVAL), np.float32(0.0))
    return {
        "Wqk": Wqk_h.astype(BF),
        "Wv": np.ascontiguousarray(W_attn[:, 2 * D:3 * D]).astype(BF),
        "Wp": np.ascontiguousarray(W_proj).astype(BF),
        "bqk": bqk_h,
        "bvrow": bv_h.astype(BF),
        "bprow": np.asarray(b_proj).reshape(1, D).astype(BF),
        "dlut": dlut_h.astype(BF),
        "dlv": dlv_h.astype(BF),
        "zbT": zbT_h.astype(BF),
    }


def kernel(x, W_attn, b_attn, W_proj, b_proj, lut_k, lut_v):
    x = np.asarray(x, np.float32)
    shared = _host_prep(np.asarray(W_attn, np.float32),
                        np.asarray(b_attn, np.float32),
                        np.asarray(W_proj, np.float32),
                        np.asarray(b_proj, np.float32),
                        np.asarray(lut_k, np.float32),
                        np.asarray(lut_v, np.float32))
    if "nc" not in _CACHE:
        _CACHE["nc"] = build_module()
    nc = _CACHE["nc"]
    in_maps = []
    for b in range(N_CORES):
        m = dict(shared)
        m["xT"] = np.ascontiguousarray(x[b].T).astype(BF)
        in_maps.append(m)
    res = run_bass_kernel_spmd(nc, in_maps, list(range(N_CORES)), trace=TRACE)
    _CACHE["last_result"] = res
    out = np.stack([res.results[b]["OUT"] for b in range(N_CORES)], axis=0)
    return out.astype(np.float32)


# revision 21
# speedup vs baseline: 1.9243x; 1.0862x over previous
import sys

sys.path.insert(0, "/opt/trn_rl_repo")

import math

import numpy as np
import ml_dtypes

import concourse.bass as bass
import concourse.mybir as mybir
import concourse.tile as tile
from concourse import bacc
from concourse.bass_utils import run_bass_kernel_spmd
from concourse.masks import make_identity

F32 = mybir.dt.float32
F32R = mybir.dt.float32r
BF16 = mybir.dt.bfloat16
EXPF = mybir.ActivationFunctionType.Exp
RECIPF = mybir.ActivationFunctionType.Reciprocal

B, S, D = 8, 1024, 1024
N_H = 16
REL_K = 16
d_k = D // N_H  # 64
N_CORES = 8
MASKVAL = -1e30
BF = np.dtype(ml_dtypes.bfloat16)

_CACHE = {}
TRACE = False


def build_module():
    nc = bacc.Bacc("TRN2", detect_race_conditions=False, num_swdge_queues=4)

    xT = nc.dram_tensor("xT", [D, S], BF16, kind="ExternalInput")
    Wqk = nc.dram_tensor("Wqk", [D, 2 * D], BF16, kind="ExternalInput")
    Wv = nc.dram_tensor("Wv", [D, D], BF16, kind="ExternalInput")
    Wp = nc.dram_tensor("Wp", [D, D], BF16, kind="ExternalInput")
    bqk = nc.dram_tensor("bqk", [128, 16], F32, kind="ExternalInput")
    bvrow = nc.dram_tensor("bvrow", [1, D], BF16, kind="ExternalInput")
    bprow = nc.dram_tensor("bprow", [1, D], BF16, kind="ExternalInput")
    dlut = nc.dram_tensor("dlut", [d_k, 16], BF16, kind="ExternalInput")
    dlv = nc.dram_tensor("dlv", [16, d_k], BF16, kind="ExternalInput")
    zbT = nc.dram_tensor("zbT", [128, 2561], BF16, kind="ExternalInput")
    OUT = nc.dram_tensor("OUT", [S, D], F32, kind="ExternalOutput")

    zbP = [nc.dram_tensor(f"zbP{k}", [128, 2561], BF16) for k in range(2)]
    dtD = [nc.dram_tensor(f"dtD{k}", [128, 160], BF16) for k in range(4)]
    ewP = [nc.dram_tensor(f"ewP{k}", [128, 1153], BF16) for k in range(4)]

    with tile.TileContext(nc) as tc:
        with (
            tc.tile_pool(name="pers", bufs=1) as pers,
            tc.tile_pool(name="mm", bufs=4, space="PSUM") as mmp,
        ):
            # ---- resident loads first, split across queues ----
            xT_sb = []
            for d in range(8):
                t = pers.tile([128, S], BF16, tag=f"xT{d}")
                eng = nc.sync if d % 2 == 0 else nc.scalar
                eng.dma_start(out=t[:], in_=xT[128 * d:128 * (d + 1), :])
                xT_sb.append(t)

            # ---- constants ----
            identf = pers.tile([128, 128], F32)
            make_identity(nc, identf[:])
            identb = pers.tile([128, 128], BF16)
            nc.vector.tensor_copy(identb[:], identf[:])
            dlut_sb = pers.tile([128, 16], BF16)
            nc.scalar.dma_start(out=dlut_sb[0:64, :], in_=dlut[:])
            nc.scalar.dma_start(out=dlut_sb[64:128, :], in_=dlut[:])
            dlv_sb = pers.tile([16, d_k], BF16)
            nc.scalar.dma_start(out=dlv_sb[:], in_=dlv[:])
            bqk_sb = pers.tile([128, 16], F32)
            nc.sync.dma_start(out=bqk_sb[:], in_=bqk[:])
            bv_sb = pers.tile([1, D], BF16)
            nc.sync.dma_start(out=bv_sb[:], in_=bvrow[:])
            bp_sb = pers.tile([1, D], BF16)
            nc.sync.dma_start(out=bp_sb[:], in_=bprow[:])
            ones_row = pers.tile([1, 512], BF16)
            nc.vector.memset(ones_row[:], 1.0)
            ones_col = pers.tile([1, 128], BF16)
            nc.vector.memset(ones_col[:], 1.0)
            ones_rf = pers.tile([1, 64], F32)
            nc.vector.memset(ones_rf[:], 1.0)
            ones_r = pers.tile([1, 64], F32R)
            nc.vector.tensor_copy(ones_r[:], ones_rf[:])

            # ---- v projection -> vhat_sb (65-stride layout + ones cols) ----
            vhat_sb = [pers.tile([128, 16 * 65], BF16, name=f"vh{jt}", tag=f"vh{jt}")
                       for jt in range(8)]
            with tc.tile_pool(name="wv", bufs=1) as wvp:
                # zbP init (template with causal mask / zeros); lives in the wv
                # pool so its teardown barrier lands after the v phase.
                zb_sb = wvp.tile([128, 2561], BF16)
                nc.sync.dma_start(out=zb_sb[:], in_=zbT[:])
                for k in range(2):
                    nc.sync.dma_start(out=bass.AP(tensor=zbP[k], offset=0,
                                                  ap=[[2561, 128], [1, 2561]]),
                                      in_=zb_sb[:])
                Wv_sb = []
                for d in range(8):
                    t = wvp.tile([128, D], BF16, tag=f"wv{d}")
                    nc.gpsimd.dma_start(out=t[:], in_=Wv[128 * d:128 * (d + 1), :])
                    Wv_sb.append(t)
                # Wqk on gpsimd, overlaps the v-projection compute
                Wqk_sb = []
                for d in range(8):
                    t = pers.tile([128, 2 * D], BF16, name=f"wqk{d}", tag=f"wqk{d}")
                    nc.gpsimd.dma_start(out=t[:], in_=Wqk[128 * d:128 * (d + 1), :])
                    Wqk_sb.append(t)
                for tt in range(8):
                    vt = vhat_sb[tt]
                    ones_ap = bass.AP(tensor=vt[:].tensor, offset=64,
                                      ap=[[16 * 65, 128], [65, 16]])
                    nc.vector.memset(ones_ap, 1.0)
                    for fc in range(2):
                        ps = mmp.tile([128, 512], F32, tag="mm")
                        for d in range(8):
                            nc.tensor.matmul(
                                ps[:],
                                xT_sb[d][:, 128 * tt:128 * (tt + 1)],
                                Wv_sb[d][:, 512 * fc:512 * (fc + 1)],
                                start=(d == 0), stop=False,
                            )
                        nc.tensor.matmul(
                            ps[:], ones_col[:],
                            bv_sb[:, 512 * fc:512 * (fc + 1)],
                            start=False, stop=True,
                        )
                        srcA = bass.AP(tensor=ps[:].tensor,
                                       offset=ps[:].offset,
                                       ap=[[512, 128], [64, 8], [1, 64]])
                        dst = bass.AP(tensor=vt[:].tensor, offset=65 * 8 * fc,
                                      ap=[[16 * 65, 128], [65, 8], [1, 64]])
                        nc.scalar.copy(dst, srcA)

            pair_sb = [pers.tile([128, S], BF16, name=f"pair{hp}", tag=f"pair{hp}")
                       for hp in range(8)]
            Wp_sb = [pers.tile([128, D], BF16, name=f"wp{d}", tag=f"wp{d}")
                     for d in range(8)]

            # dt strips go through DRAM: the diagonal write covers the same
            # cells every pair; the zero background persists in DRAM.
            dtz = pers.tile([128, 160], BF16)
            nc.vector.memset(dtz[:], 0.0)
            for k2 in range(4):
                nc.gpsimd.dma_start(out=dtD[k2][:], in_=dtz[:])

            expT = pers.tile([128, 16 * 1024], BF16)
            # block-7 tail cols [128:144) per head are read by the ewP write but
            # never written by exps -> zero them once.
            nc.vector.memset(expT[:, 2048 * 7 + 128:2048 * 7 + 144], 0.0)
            nc.vector.memset(expT[:, 2048 * 7 + 1024 + 128:2048 * 7 + 1024 + 144], 0.0)

            # ---- attention ----
            with (
                tc.tile_pool(name="qk", bufs=2) as qkp,
                tc.tile_pool(name="dpp", bufs=2) as dpp,
                tc.tile_pool(name="bandp", bufs=2) as bandp,
                tc.tile_pool(name="eskp", bufs=2) as eskp,
                tc.tile_pool(name="denp", bufs=1) as denp,
                tc.tile_pool(name="pso", bufs=1, space="PSUM") as psop,
            ):
                def emit_qkproj_sec(hp, sec):
                    ftbase = 1024 * sec + 128 * hp
                    ft = 8 * sec + hp
                    dstt = qkp.tile([128, S], BF16, name=f"qk{sec}", tag=f"qk{sec}")
                    for tch in range(2):
                        ps = mmp.tile([128, 512], F32, tag="mm")
                        for d in range(8):
                            nc.tensor.matmul(
                                ps[:],
                                Wqk_sb[d][:, ftbase:ftbase + 128],
                                xT_sb[d][:, 512 * tch:512 * (tch + 1)],
                                start=(d == 0), stop=(d == 7),
                            )
                        nc.vector.tensor_scalar_add(
                            dstt[:, 512 * tch:512 * (tch + 1)], ps[:],
                            bqk_sb[:, ft:ft + 1])
                    return dstt

                def emit_band_part1(hp, qT_pair):
                    """dp strip matmuls + sheared dpSh DMA."""
                    dpT_h = []
                    for h in range(2):
                        dpT = dpp.tile([16, 1040], BF16, name=f"dpT{h}", tag=f"dpT{h}")
                        nc.vector.memset(dpT[:, 1024:1040], 0.0)
                        for tch in range(2):
                            psdp = mmp.tile([16, 512], F32, tag="mm")
                            nc.tensor.matmul(psdp[0:16, :],
                                             dlut_sb[64 * h:64 * h + 64, :],
                                             qT_pair[64 * h:64 * h + 64,
                                                     512 * tch:512 * (tch + 1)],
                                             start=True, stop=True)
                            nc.vector.tensor_copy(dpT[:, 512 * tch:512 * (tch + 1)],
                                                  psdp[0:16, :])
                        dpT_h.append(dpT)
                    dpSh = dpp.tile([32, 1024], BF16, tag="dpSh")
                    for h in range(2):
                        src = bass.AP(tensor=dpT_h[h][:].tensor, offset=0,
                                      ap=[[1041, 16], [1, 1024]])
                        nc.sync.dma_start(out=dpSh[16 * h:16 * h + 16, :], in_=src)
                    return dpSh

                def emit_band_part2(hp, dpSh):
                    """strip transposes + batched band write / masked read-back."""
                    dpS_all = dpp.tile([128, 256], BF16, tag="dpS")
                    for jt in range(8):
                        psd = mmp.tile([128, 32], BF16, tag="mm")
                        nc.tensor.matmul(psd[:, 0:32], dpSh[:, 128 * jt:128 * (jt + 1)],
                                         identb[0:32, 0:32], is_transpose=True,
                                         skip_group_check=True)
                        nc.vector.tensor_copy(dpS_all[:, 32 * jt:32 * (jt + 1)],
                                              psd[:, 0:32])
                    zb = zbP[hp % 2]
                    dstW = bass.AP(tensor=zb, offset=0,
                                   ap=[[2562, 128], [160, 16], [1, 16]])
                    srcW = bass.AP(tensor=dpS_all[:].tensor, offset=0,
                                   ap=[[256, 128], [16, 16], [1, 16]])
                    nc.scalar.dma_start(out=dstW, in_=srcW)
                    band = bandp.tile([128, 2560], BF16, tag="band")
                    srcR = bass.AP(tensor=zb, offset=0,
                                   ap=[[2561, 128], [1, 2560]])
                    nc.sync.dma_start(out=band[:], in_=srcR)
                    return band

                def emit_scores_head(hp, h, qT_pair, kT_pair, band, pso,
                                     jts=range(8)):
                    hg = 2 * hp + h
                    qT = qT_pair[64 * h:64 * h + 64, :]
                    kT = kT_pair[64 * h:64 * h + 64, :]
                    for jt in jts:
                        j0 = 128 * jt
                        wd = min(512, S - j0)
                        win = min(144, S - j0)
                        ebase = 2048 * jt + 1024 * h
                        pss = mmp.tile([128, 512], F32, tag="mm")
                        nc.tensor.matmul(pss[:, 0:wd],
                                         kT[:, j0:j0 + 128],
                                         qT[:, j0:j0 + wd],
                                         start=True, stop=True)
                        nc.vector.tensor_add(
                            pss[:, 0:win], pss[:, 0:win],
                            band[:, 320 * jt + 160 * h:320 * jt + 160 * h + win])
                        nc.scalar.activation(expT[:, ebase:ebase + wd],
                                             pss[:, 0:wd], EXPF)
                        if S - j0 > 512:
                            w1 = S - j0 - 512
                            pss1 = mmp.tile([128, 512], F32, tag="mm")
                            nc.tensor.matmul(pss1[:, 0:w1],
                                             kT[:, j0:j0 + 128],
                                             qT[:, j0 + 512:S],
                                             start=True, stop=True)
                            nc.scalar.activation(expT[:, ebase + 512:ebase + 512 + w1],
                                                 pss1[:, 0:w1], EXPF)
                        lhs = vhat_sb[jt][:, 65 * hg:65 * hg + 65]
                        segs = ([(j0, 512), (512, 1024)] if j0 < 512
                                else [(j0, 1024)])
                        for (a, b2) in segs:
                            nc.tensor.matmul(pso[:, a:b2], lhs,
                                             expT[:, ebase + a - j0:ebase + b2 - j0],
                                             start=(jt == 0), stop=False,
                                             skip_group_check=True)

                # prologue: pair 0 qk + band chain
                qk_q0 = emit_qkproj_sec(0, 0)
                dpSh0 = emit_band_part1(0, qk_q0)
                qk_next = [qk_q0, emit_qkproj_sec(0, 1)]
                band_next = emit_band_part2(0, dpSh0)
                qk_q = emit_qkproj_sec(1, 0)
                dpSh_next = emit_band_part1(1, qk_q)

                dt_tiles = [None, None]
                recip_h = [None, None]

                def emit_recip(h):
                    rec = denp.tile([1, 1024], F32R, name=f"rec{h}", tag=f"rec{h}")
                    with nc.allow_low_precision(reason="denom recip broadcast"):
                        nc.vector.reciprocal(rec[:], pso_h[h][64:65, :])
                    recip_h[h] = rec

                def emit_esk(hp, h):
                    ew = ewP[2 * (hp % 2) + h]
                    dstE = bass.AP(tensor=ew, offset=0,
                                   ap=[[1152, 128], [144, 8], [1, 144]])
                    srcE = bass.AP(tensor=expT[:].tensor, offset=1024 * h,
                                   ap=[[16384, 128], [2048, 8], [1, 144]])
                    nc.scalar.dma_start(out=dstE, in_=srcE)
                    esk = eskp.tile([128, 128], BF16, name=f"esk{h}", tag=f"esk{h}")
                    srcK = bass.AP(tensor=ew, offset=0,
                                   ap=[[1153, 128], [144, 8], [1, 16]])
                    nc.sync.dma_start(out=esk[:], in_=srcK)
                    return esk

                def emit_strip_chain(hp, h, esk):
                    psT = mmp.tile([128, 128], BF16, tag="mm")
                    nc.tensor.matmul(psT[:, :], esk[:], identb[:, :],
                                     is_transpose=True, skip_group_check=True)
                    eskT = eskp.tile([128, 128], BF16, name=f"eskT{h}",
                                     tag=f"eskT{h}")
                    nc.vector.tensor_copy(eskT[:], psT[:, :])
                    dtd = dtD[2 * (hp % 2) + h]
                    eng = nc.sync if h == 0 else nc.scalar
                    eng.dma_start(out=bass.AP(tensor=dtd, offset=0,
                                              ap=[[160, 128], [1, 128]]),
                                  in_=eskT[:])
                    dt_t = eskp.tile([16, 1280], BF16, name=f"dt{h}", tag=f"dt{h}")
                    srcR = bass.AP(tensor=dtd, offset=0,
                                   ap=[[159, 16], [2560, 8], [1, 160]])
                    dstR = bass.AP(tensor=dt_t[:].tensor, offset=0,
                                   ap=[[1280, 16], [160, 8], [1, 160]])
                    eng.dma_start(out=dstR, in_=srcR)
                    dt_tiles[h] = dt_t

                def emit_psb(h):
                    psb_cs = []
                    for c in range(2):
                        psb = mmp.tile([64, 512], F32, tag="mm")
                        nc.tensor.matmul(psb[0:64, :], ones_r[:],
                                         recip_h[h][:, 512 * c:512 * (c + 1)],
                                         start=True, stop=True)
                        psb_cs.append(psb)
                    return psb_cs

                def emit_dlv(h):
                    pso = pso_h[h]
                    for jt in range(8):
                        j0 = 128 * jt
                        win = min(144, S - j0)
                        a0, b0 = j0, j0 + win
                        dsegs = ([(a0, 512), (512, b0)] if (a0 < 512 < b0)
                                 else [(a0, b0)])
                        for (a, b2) in dsegs:
                            nc.tensor.matmul(
                                pso[0:64, a:b2], dlv_sb[:],
                                dt_tiles[h][:, 160 * jt + a - j0:160 * jt + b2 - j0],
                                start=False,
                                stop=(jt == 7 and (a, b2) == dsegs[-1]),
                                skip_group_check=True)

                def emit_evict_mul(hp, h, psb_cs):
                    for c in range(2):
                        nc.scalar.copy(
                            pair_sb[hp][64 * h:64 * h + 64, 512 * c:512 * (c + 1)],
                            pso_h[h][0:64, 512 * c:512 * (c + 1)])
                        nc.vector.tensor_mul(
                            pair_sb[hp][64 * h:64 * h + 64, 512 * c:512 * (c + 1)],
                            pair_sb[hp][64 * h:64 * h + 64, 512 * c:512 * (c + 1)],
                            psb_cs[c][0:64, :])

                # prologue covers pair 0; loop prefetches pair p+1's k-proj /
                # pair p+2's q-proj inside pair p's tail as stall filler.
                qk_k = qk_next[1]
                for hp in range(8):
                    qT_pair, kT_pair = qk_next
                    band = band_next
                    pso_h = [psop.tile([65, 1024], F32, name=f"pso{h}", tag=f"pso{h}")
                             for h in range(2)]
                    emit_scores_head(hp, 0, qT_pair, kT_pair, band, pso_h[0])
                    esk0 = emit_esk(hp, 0)
                    if hp < 7:
                        band_next = emit_band_part2(hp + 1, dpSh_next)
                    emit_scores_head(hp, 1, qT_pair, kT_pair, band, pso_h[1])
                    esk1 = emit_esk(hp, 1)
                    emit_recip(0)
                    emit_recip(1)
                    emit_strip_chain(hp, 0, esk0)
                    if hp < 7:
                        qk_k = emit_qkproj_sec(hp + 1, 1)
                    psb0 = emit_psb(0)
                    emit_dlv(0)
                    emit_evict_mul(hp, 0, psb0)
                    emit_strip_chain(hp, 1, esk1)
                    if hp < 7:
                        qk_next = [qk_q, qk_k]
                        if hp < 6:
                            qk_q = emit_qkproj_sec(hp + 2, 0)
                            dpSh_next = emit_band_part1(hp + 2, qk_q)
                    psb1 = emit_psb(1)
                    emit_dlv(1)
                    emit_evict_mul(hp, 1, psb1)
                    if hp == 3:
                        for d in range(8):
                            nc.gpsimd.dma_start(out=Wp_sb[d][:],
                                                in_=Wp[128 * d:128 * (d + 1), :])

            # ---- final projection ----
            with (
                tc.tile_pool(name="ps_p", bufs=2, space="PSUM") as ps_p,
                tc.tile_pool(name="outp", bufs=2) as outp,
            ):
                for tt in range(8):
                    ps = ps_p.tile([128, 1024], F32, tag="psp")
                    for fc in range(2):
                        for d in range(8):
                            nc.tensor.matmul(
                                ps[:, 512 * fc:512 * (fc + 1)],
                                pair_sb[d][:, 128 * tt:128 * (tt + 1)],
                                Wp_sb[d][:, 512 * fc:512 * (fc + 1)],
                                start=(d == 0), stop=False,
                            )
                        nc.tensor.matmul(
                            ps[:, 512 * fc:512 * (fc + 1)],
                            ones_col[:],
                            bp_sb[:, 512 * fc:512 * (fc + 1)],
                            start=False, stop=True,
                        )
                    ot = outp.tile([128, 1024], F32, tag="ot")
                    nc.vector.tensor_copy(ot[:], ps[:])
                    nc.sync.dma_start(out=OUT[128 * tt:128 * (tt + 1), :], in_=ot[:])

    nc.compile()
    return nc


def _host_prep(W_attn, b_attn, W_proj, b_proj, lut_k, lut_v):
    scale = 1.0 / math.sqrt(d_k)
    Wqk_h = np.concatenate([W_attn[:, :D], W_attn[:, D:2 * D] * scale], axis=1)
    bq = b_attn[:D]
    bk = b_attn[D:2 * D] * scale
    bqk_h = np.stack([np.concatenate([bq, bk])[128 * ft:128 * (ft + 1)]
                      for ft in range(16)], axis=1).astype(np.float32)
    bv_h = (b_attn[2 * D:3 * D] + np.tile(lut_v[0], N_H)).reshape(1, D)
    dlut_h = np.stack([(lut_k[16 - u] - lut_k[0]) * scale for u in range(16)],
                      axis=1)
    dlv_h = np.stack([lut_v[16 - u] - lut_v[0] for u in range(16)], axis=0)
    # zbP template: flat[2561*p + col], col = 160*jh + c (16 blocks of 160):
    # c < p -> MASKVAL (causal), c >= p+16 -> 0; band cells get overwritten.
    c_of_col = (np.arange(2561) % 160)[None, :]
    pvec = np.arange(128)[:, None]
    zbT_h = np.where(c_of_col < pvec, np.float32(MASKVAL), np.float32(0.0))
    return {
        "Wqk": Wqk_h.astype(BF),
        "Wv": np.ascontiguousarray(W_attn[:, 2 * D:3 * D]).astype(BF),
        "Wp": np.ascontiguousarray(W_proj).astype(BF),
        "bqk": bqk_h,
        "bvrow": bv_h.astype(BF),
        "bprow": np.asarray(b_proj).reshape(1, D).astype(BF),
        "dlut": dlut_h.astype(BF),
        "dlv": dlv_h.astype(BF),
        "zbT": zbT_h.astype(BF),
    }


def kernel(x, W_attn, b_attn, W_proj, b_proj, lut_k, lut_v):
    x = np.asarray(x, np.float32)
    shared = _host_prep(np.asarray(W_attn, np.float32),
                        np.asarray(b_attn, np.float32),
                        np.asarray(W_proj, np.float32),
                        np.asarray(b_proj, np.float32),
                        np.asarray(lut_k, np.float32),
                        np.asarray(lut_v, np.float32))
    if "nc" not in _CACHE:
        _CACHE["nc"] = build_module()
    nc = _CACHE["nc"]
    in_maps = []
    for b in range(N_CORES):
        m = dict(shared)
        m["xT"] = np.ascontiguousarray(x[b].T).astype(BF)
        in_maps.append(m)
    res = run_bass_kernel_spmd(nc, in_maps, list(range(N_CORES)), trace=TRACE)
    _CACHE["last_result"] = res
    out = np.stack([res.results[b]["OUT"] for b in range(N_CORES)], axis=0)
    return out.astype(np.float32)
